# revision 36
# baseline (speedup 1.0000x reference)
"""Trainium2 Bass kernel for nn_Attention (Bahdanau-style additive attention).

Reference computation:
    enc = encoder_outputs.transpose(1, 0, 2)            # [B, S, 2H]
    e_proj = enc @ w_e.T                                # [B, S, H]
    energy = tanh(h_proj[:, None, :] + e_proj + b)      # [B, S, H]
    att = energy @ v_w                                  # [B, S]
    out = softmax(att, axis=1)

Sharding: data-parallel over batch, 4 batch rows per core on 8 cores.

Per-core pipeline, |v|-stratified mixed precision: the logit error from
quantizing the e_proj GEMM is sum_h v_h * tanh' * dx_h, so the h
columns are permuted by descending |v_h| (host side) and the HOT
highest-|v| columns are computed in fp16 while the remaining COLD
columns run entirely in fp8 (e4m3) DoubleRow matmuls at 2x PE
throughput.  HOT=192 puts the hot phase right at the fp16 LdWeights
pipeline floor (~83ns/chunk), so shrinking it buys no time and growing
it costs streaming cycles; the steady state runs at the PE roofline
(~4.2us per 128-position s-tile).
  - enc is pre-transposed and quantized on the host into quarter-major
    [b, q, e, chunk, s(512)] lines and streamed per quarter on the SP
    DMA queue with 2KB descriptors (giant per-partition descriptors
    measure ~2x slower per byte; <=512B lines lose the byte-weighted
    engine arbitration against bigger concurrent descriptors)
  - startup: engines wake ~5us in, dynamic DMA queues only ~8us and
    ramp slowly until ~25us.  Dep-free dummy matmuls warm the PE (HAM
    K=8/8) until real data lands; w16/w8 are loaded in chunk groups so
    subtile deps release the first matmuls early; quarter 0 issues all
    four hot phases before any cold phase so the PE FIFO never blocks
    on the later-arriving e8/w8
  - w_e and c_b are pre-scaled by WS=64 so the fp8 weights stay in the
    e4m3 normal range; the tanh activation applies scale=1/WS to undo it
  - s-tiles are processed in pairs (hot,hot,cold,cold) to halve the
    hot<->cold transitions whose DR-LdWeights exposure (~100ns) the
    weight-load pipeline cannot hide
  - epilogue per s-tile: DVE adds the broadcast bias c_b*WS per psum
    region, ACT applies tanh(x/WS), GPSIMD (otherwise idle) multiplies
    by v, DVE reduces over h into the logit column; the final tile's
    multiply runs on DVE (3x faster) to shorten the drain chain
h_proj ([32,1024] @ [1024,1024]) and the final softmax over [32, 2048]
are tiny and run on the host in fp32.
"""

import sys

try:
    import concourse.bass as bass  # noqa: F401
except ImportError:
    sys.path.insert(0, "/opt/trn_rl_repo")

import numpy as np
import ml_dtypes

import concourse.bacc as bacc
import concourse.mybir as mybir
import concourse.tile as tile
from concourse.bass_utils import run_bass_kernel_spmd

HID = 1024
BATCH = 32
SRC_LEN = 2048

N_CORES = 8
B_LOC = BATCH // N_CORES      # 4
E = 2 * HID                   # 2048
N_EC = E // 128               # 16 e-chunks of 128
N_DR = N_EC // 2              # 8 fp8 DoubleRow chunk-pairs
HOT = 176                     # fp16 h-columns (highest |v|), permuted first
COLD = HID - HOT              # 832 fp8 h-columns
C0 = 512                      # cold psum region split: 512 + 320
C1 = COLD - C0                # 320
SH = SRC_LEN // 2             # 1024 s per half-row pipeline stage
N_STH = SH // 128             # 8 s-tiles per half
WS = 64.0                     # weight/bias pre-scale (fp8 range)
Q = 512                       # enc streaming granularity (s-positions)
N_WARM_MM = 75                # dep-free PE warmup matmuls (N=128)

f32 = mybir.dt.float32
fp16 = mybir.dt.float16
fp8 = mybir.dt.float8e4

_NC_CACHE = {}


def _build():
    nc = bacc.Bacc(
        "TRN2", target_bir_lowering=False, debug=False, num_devices=N_CORES
    )
    # quarter-major layout: one fully-contiguous 16/8KB line per partition
    # per quarter, so each quarter load is 128 big descriptors (the DMA is
    # descriptor-rate-bound; 512B-line layouts cost ~10x the latency)
    enc16 = nc.declare_dram_parameter(
        "enc16", [B_LOC, SRC_LEN // Q, 128, N_EC, Q], fp16, isOutput=False
    )
    enc8 = nc.declare_dram_parameter(
        "enc8", [B_LOC, SRC_LEN // Q, 128, N_EC, Q], fp8, isOutput=False
    )
    w16 = nc.declare_dram_parameter("w16", [128, N_EC, HOT], fp16, isOutput=False)
    w8 = nc.declare_dram_parameter("w8", [128, N_EC, COLD], fp8, isOutput=False)
    cbb = nc.declare_dram_parameter("cbb", [B_LOC, 128, HID], f32, isOutput=False)
    vb = nc.declare_dram_parameter("vb", [128, HID], fp16, isOutput=False)
    # [b, p, st]: logit(b, st*128 + p) in permuted-h space (h only summed)
    att = nc.declare_dram_parameter(
        "att", [B_LOC, 128, SRC_LEN // 128], f32, isOutput=True
    )

    with tile.TileContext(nc) as tc:
        with (
            tc.tile_pool(name="const", bufs=1) as const_pool,
            tc.tile_pool(name="e16p", bufs=4) as e16_pool,
            tc.tile_pool(name="e8p", bufs=4) as e8_pool,
            tc.tile_pool(name="cbbp", bufs=2) as cbb_pool,
            tc.tile_pool(name="prep", bufs=4) as pre_pool,
            tc.tile_pool(name="tep", bufs=3) as te_pool,
            tc.tile_pool(name="ttp", bufs=2) as tt_pool,
            tc.tile_pool(name="attsb", bufs=1) as att_pool,
            tc.tile_pool(name="psum", bufs=2, space="PSUM") as psum_pool,
        ):
            w16_sb = const_pool.tile([128, N_EC, HOT], fp16)
            w8_sb = const_pool.tile([128, N_EC, COLD], fp8)
            vb_sb = const_pool.tile([128, HID], fp16)
            att_sb = att_pool.tile([128, BATCH // N_CORES * (SRC_LEN // 128)], f32)

            # PE warmup: engines come alive ~5us in, DMA queues only ~8us.
            # Without activity HAM holds the PE at K=4/8 (1.2 GHz) and the
            # first ~16us of real matmuls run at half rate.  Dep-free dummy
            # matmuls on an (uninitialized) tile keep the PE busy from ~5us
            # so it is warm (K=8/8) when real work arrives.
            warm_sb = const_pool.tile([128, 128], fp16)
            nc.gpsimd.memset(warm_sb[:], 0.0)
            # shares the psc0 tag so no extra PSUM bank is reserved; the
            # first real c0 tile rotates into this slot after the dummies
            warm_ps = psum_pool.tile([128, C0], f32, tag="psc0")
            for _ in range(N_WARM_MM):
                nc.tensor.matmul(
                    warm_ps[:, 0:128], lhsT=warm_sb[:], rhs=warm_sb[:],
                    start=True, stop=True,
                )

            # warmup tanh for the ACT LUT-table dependency (no DMA dep)
            warm = const_pool.tile([128, 1], f32)
            nc.scalar.activation(
                warm[:], warm_sb[:, 0:1], mybir.ActivationFunctionType.Tanh
            )

            # consts on the ACT hwdge queue, coalesced (per-partition lines
            # are contiguous, so each is ~128 big descriptors), ordered by
            # first consumption: w16 (first hot MM), w8a (first cold pair),
            # cbb0 (first bias add), w8b, vb (first reduce)
            cbb_sbs = [None] * B_LOC

            def load_cbb(b):
                t = cbb_pool.tile([128, HID], f32, tag="cbb", name=f"cbb_{b}")
                nc.scalar.dma_start(t[:], cbb[b])
                cbb_sbs[b] = t

            # w16 in 4 chunk-groups: subtile deps let the first hot MMs
            # start after the first group (192KB) instead of all 768KB
            for c0 in range(0, N_EC, 4):
                nc.scalar.dma_start(
                    w16_sb[:, c0:c0 + 4], w16[:, c0:c0 + 4]
                )
            nc.scalar.dma_start(w8_sb[:, 0:4], w8[:, 0:4])
            nc.scalar.dma_start(w8_sb[:, 4:8], w8[:, 4:8])
            load_cbb(0)
            nc.scalar.dma_start(w8_sb[:, 8:12], w8[:, 8:12])
            nc.scalar.dma_start(w8_sb[:, 12:16], w8[:, 12:16])
            nc.scalar.dma_start(vb_sb[:], vb[:])

            # stream enc in quarter-rows of Q s-positions: finer pipeline,
            # smaller SP-queue bursts, earlier first tile
            quarters = [
                (b, q) for b in range(B_LOC) for q in range(SRC_LEN // Q)
            ]
            e16_sbs = {}
            e8_sbs = {}

            def alloc_quarter(i):
                e16_sbs[i] = e16_pool.tile(
                    [128, N_EC, Q], fp16, tag="e16", name=f"e16_{i}"
                )
                e8_sbs[i] = e8_pool.tile(
                    [128, N_EC, Q], fp8, tag="e8", name=f"e8_{i}"
                )

            def load_quarter(i):
                b, q = quarters[i]
                # cap descriptors at 2KB lines: giant single-partition
                # descriptors (16KB) measure ~2x slower per byte
                nc.sync.dma_start(
                    e16_sbs[i][:], enc16[b, q], max_dma_last_dim=1024
                )
                nc.sync.dma_start(
                    e8_sbs[i][:], enc8[b, q], max_dma_last_dim=2048
                )

            # first quarter, interleaved for earliest consumption: hot
            # needs e16[s 0:128] of all chunks; cold pair j needs e8
            # chunks 2j:2j+2.  e8 is split by chunk groups (keeps 2KB
            # lines; s-slabs of fp8 would be 256B and get starved in the
            # byte-weighted engine arbitration).
            alloc_quarter(0)
            nc.sync.dma_start(
                e16_sbs[0][:, :, 0:256], enc16[0, 0, :, :, 0:256]
            )
            nc.sync.dma_start(e8_sbs[0][:, 0:4], enc8[0, 0, :, 0:4])
            nc.sync.dma_start(
                e16_sbs[0][:, :, 256:512], enc16[0, 0, :, :, 256:512]
            )
            nc.sync.dma_start(e8_sbs[0][:, 4:16], enc8[0, 0, :, 4:16])
            alloc_quarter(1)
            load_quarter(1)
            alloc_quarter(2)
            load_quarter(2)

            # split LdWeights/Matmult: the next stationary (an enc chunk)
            # loads while the current moving phase streams
            def mm(lhs, psum, rhs, start, stop, perf_mode=None):
                inst = nc.tensor.matmul(
                    psum, lhsT=lhs, rhs=rhs,
                    start=start, stop=stop, perf_mode=perf_mode,
                )
                inst.ins.ldweights = False

            def emit_hot(i, st, ps_h):
                sl = slice(st * 128, (st + 1) * 128)
                for c in range(N_EC):
                    lhs = e16_sbs[i][:, c, sl]
                    nc.tensor.ldweights(lhs)
                    mm(lhs, ps_h[:], w16_sb[:, c], start=(c == 0),
                       stop=(c == N_EC - 1))

            def emit_cold(i, st, ps_c0, ps_c1):
                sl = slice(st * 128, (st + 1) * 128)
                for j in range(N_DR):
                    lhs = e8_sbs[i][:, 2 * j:2 * j + 2, sl]
                    nc.tensor.ldweights(
                        lhs, perf_mode=mybir.MatmulPerfMode.DoubleRow
                    )
                    mm(lhs, ps_c0[:], w8_sb[:, 2 * j:2 * j + 2, 0:C0],
                       start=(j == 0), stop=(j == N_DR - 1),
                       perf_mode=mybir.MatmulPerfMode.DoubleRow)
                    mm(lhs, ps_c1[:], w8_sb[:, 2 * j:2 * j + 2, C0:COLD],
                       start=(j == 0), stop=(j == N_DR - 1),
                       perf_mode=mybir.MatmulPerfMode.DoubleRow)

            def emit_epilogue(i, b, q, st, ps_h, ps_c0, ps_c1, last):
                tanhE = te_pool.tile(
                    [128, HID], fp16, tag="te", name=f"te_{i}_{st}"
                )
                for ps, lo, hi in (
                    (ps_h, 0, HOT),
                    (ps_c0, HOT, HOT + C0),
                    (ps_c1, HOT + C0, HID),
                ):
                    pre = pre_pool.tile(
                        [128, hi - lo], f32, tag="pre",
                        name=f"pre_{i}_{st}_{lo}",
                    )
                    nc.vector.tensor_add(
                        out=pre[:], in0=ps[:], in1=cbb_sbs[b][:, lo:hi]
                    )
                    nc.scalar.activation(
                        tanhE[:, lo:hi], pre[:],
                        mybir.ActivationFunctionType.Tanh,
                        scale=1.0 / WS,
                    )
                tt = tt_pool.tile(
                    [128, HID], fp16, tag="tt", name=f"tt_{i}_{st}"
                )
                # final tile: mul on DVE (3x faster than GpSimd) to
                # shorten the drain chain; elsewhere GpSimd so the DVE
                # stays under its per-tile budget
                mul_eng = nc.vector if last else nc.gpsimd
                mul_eng.tensor_mul(out=tt[:], in0=tanhE[:], in1=vb_sb[:])
                k = b * (SRC_LEN // 128) + q * (Q // 128) + st
                nc.vector.tensor_reduce(
                    att_sb[:, k:k + 1],
                    tt[:],
                    mybir.AxisListType.X,
                    mybir.AluOpType.add,
                )

            # s-tiles processed in groups: hot phases first, then cold
            # phases, then epilogues.  This halves (pairs) or quarters
            # (quarter 0) the hot<->cold phase transitions, whose
            # DR-LDWEIGHTS exposure (~100ns) the weight-load pipeline
            # cannot hide.  Quarter 0 uses one group of 4 so ALL its hot
            # work (needing only the earliest-arriving e16+w16) can run
            # before the PE queue blocks on the later e8/w8 arrivals.
            def emit_group(i, b, q, sts):
                ps = {}
                for st in sts:
                    ps[st] = (
                        psum_pool.tile(
                            [128, HOT], f32, tag="psh",
                            name=f"psh_{i}_{st}", bufs=4,
                        ),
                        psum_pool.tile(
                            [128, C0], f32, tag="psc0",
                            name=f"psc0_{i}_{st}"
                        ),
                        psum_pool.tile(
                            [128, C1], f32, tag="psc1",
                            name=f"psc1_{i}_{st}"
                        ),
                    )
                for st in sts:
                    emit_hot(i, st, ps[st][0])
                for st in sts:
                    emit_cold(i, st, ps[st][1], ps[st][2])
                for st in sts:
                    last = (i == len(quarters) - 1
                            and st == Q // 128 - 1)
                    emit_epilogue(i, b, q, st, *ps[st], last)

            for i, (b, q) in enumerate(quarters):
                groups = ((0, 1, 2, 3),) if i == 0 else ((0, 1), (2, 3))
                for sts in groups:
                    if i + 3 < len(quarters) and sts[0] == 0:
                        alloc_quarter(i + 3)
                        load_quarter(i + 3)
                    if q == 2 and sts[0] == 0 and b + 1 < B_LOC:
                        load_cbb(b + 1)
                    emit_group(i, b, q, sts)
                if q == SRC_LEN // Q - 1:
                    nst = SRC_LEN // 128
                    nc.scalar.dma_start(
                        att[b], att_sb[:, b * nst:(b + 1) * nst]
                    )
    nc.compile()
    return nc


def _get_nc():
    if "nc" not in _NC_CACHE:
        _NC_CACHE["nc"] = _build()
    return _NC_CACHE["nc"]


def kernel(hidden, encoder_outputs, attn_w, attn_b, v_w, _trace=False):
    hidden = np.asarray(hidden, dtype=np.float32)
    encoder_outputs = np.asarray(encoder_outputs, dtype=np.float32)
    attn_w = np.asarray(attn_w, dtype=np.float32)
    attn_b = np.asarray(attn_b, dtype=np.float32)
    v_w = np.asarray(v_w, dtype=np.float32)

    perm = np.argsort(-np.abs(v_w))                    # hot |v| first
    c_b = ((hidden @ attn_w[:, :HID].T + attn_b)[:, perm]) * WS
    w_e = attn_w[:, HID:][perm]                        # [H, E] permuted rows
    # [E, H] -> [chunk, e, h] -> partition-major [e, chunk, h]
    w_t = np.ascontiguousarray(
        (w_e.T * WS).reshape(N_EC, 128, HID).transpose(1, 0, 2)
    )
    w16_dev = w_t[:, :, :HOT].astype(np.float16)
    w8_dev = np.ascontiguousarray(w_t[:, :, HOT:]).astype(ml_dtypes.float8_e4m3)
    vb_dev = np.ascontiguousarray(
        np.broadcast_to(v_w[perm][None, :], (128, HID))
    ).astype(np.float16)

    nc = _get_nc()
    in_maps = []
    nq = SRC_LEN // Q
    for core in range(N_CORES):
        b0 = core * B_LOC
        e16_rows = np.empty((B_LOC, nq, 128, N_EC, Q), dtype=np.float16)
        e8_rows = np.empty((B_LOC, nq, 128, N_EC, Q), dtype=ml_dtypes.float8_e4m3)
        for b in range(B_LOC):
            # enc[:, b, :] is [S, E]; make [quarter, e, chunk, s] lines
            ect = encoder_outputs[:, b0 + b, :].T.reshape(N_EC, 128, nq, Q)
            ect = ect.transpose(2, 1, 0, 3)
            e16_rows[b] = ect
            e8_rows[b] = ect
        cbb_dev = np.ascontiguousarray(
            np.broadcast_to(c_b[b0:b0 + B_LOC, None, :], (B_LOC, 128, HID))
        ).astype(np.float32)
        in_maps.append(
            {
                "enc16": e16_rows,
                "enc8": e8_rows,
                "w16": w16_dev,
                "w8": w8_dev,
                "cbb": cbb_dev,
                "vb": vb_dev,
            }
        )

    res = run_bass_kernel_spmd(
        nc, in_maps, core_ids=list(range(N_CORES)), trace=_trace
    )
    if _trace:
        _NC_CACHE["last_result"] = res

    att = np.concatenate(
        [
            res.results[c]["att"].transpose(0, 2, 1).reshape(B_LOC, SRC_LEN)
            for c in range(N_CORES)
        ],
        axis=0,
    )  # [B, S] logits

    m = att.max(axis=1, keepdims=True)
    e = np.exp(att - m)
    out = e / e.sum(axis=1, keepdims=True)
    return out.astype(np.float32)

